# revision 36
# baseline (speedup 1.0000x reference)
"""Trainium2 Bass kernel for nn_DotAtt_10179072492025.

reference:
    scores  = einsum('bqd,bkd->bqk', q, k) * 64.0        # [B, Lq, Lk]
    weights = softmax(scores, axis=-2)                   # over the QUERY axis
    keep    = bernoulli(key(42), 0.9, weights.shape)
    weights = where(keep, weights / 0.9, 0)
    out     = einsum('bqk,bkd->bqd', weights, v)         # [B, Lq, D]

Strategy (B=16 sharded across 8 cores, 2 batches/core, no cross-core comm):
  Work in the TRANSPOSED score layout S_T[k, q] so the softmax reduction axis
  (q) is the free axis.  Scores need ~fp32 precision (SCALE=64 makes exp
  amplify any score error), but fp32 matmul is 4 cyc/row and float32r is only
  ~12-bit on HW; instead q and k are split hi/lo in bf16 and stacked along
  the contract dim (2*64=128, full PE array).  Two bf16 matmuls per output
  slice yield all four cross terms:
    MM1: [hi_k;lo_k]^T @ [hi_q;lo_q]  = hh + ll
    MM2: [hi_k;lo_k]^T @ [lo_q;hi_q]  = hl + lh
  Per k-tile of 128 rows:
    PE  : S_T[128,2048] psum = the two matmuls per 512-slice (accumulated)
    DVE : negmax[k] = -max_q S_T               (reduce_max, negate)
    ACT : W[k,q] = exp(S_T + negmax) -> bf16, sum[k] = accum_out (one op)
    DVE : W *= mask (in-place bf16 tensor_tensor, 2x perf mode)
    ACT : v'[k,:] = v[k,:] * (1/sum[k])        (Copy activation, scale=AP)
    PE  : outT[d,q] += v'.T @ W                (phase 2, accumulate over k)
  Host side: q pre-scaled by 64 (exact, pow2), v pre-scaled by 1/0.9 and cast
  bf16, dropout mask from jax threefry on CPU (bit-exact with the reference),
  transposed to [k,q] bf16 {0,1}; output returned transposed and fixed on host.
  Built with bacc.Bacc (its compile() legalizes to <=1 sync wait per
  instruction, a TRN2 hardware constraint this walrus build enforces).
"""

import os
import sys

if "/opt/trn_rl_repo" not in sys.path:
    sys.path.insert(0, "/opt/trn_rl_repo")

import numpy as np

B, LQ, LK, D = 16, 2048, 2048, 64
SCALE = 64.0
DROP_P = 0.1
N_CORES = 8
BPC = B // N_CORES  # batches per core

# tunables (env-overridable for A/B testing)
W_DT = os.environ.get("DOTATT_W_DT", "bfloat16")  # weight tile dtype
MASK_DT = os.environ.get("DOTATT_MASK_DT", "bfloat16")  # mask dram dtype
MASK_ENGINE = os.environ.get("DOTATT_MASK_ENGINE", "gpsimd")  # gpsimd|vector
QK_DT = os.environ.get("DOTATT_QK_DT", "float32r")  # float32r | float32


def build_nc(bpc=BPC, lq=LQ, lk=LK, d=D, w_dt_name=W_DT, mask_dt_name=MASK_DT,
             qk_dt_name=QK_DT, mask_engine=MASK_ENGINE):
    """Build the Bass/Tile program (SPMD: same program on every core)."""
    import concourse.bass as bass  # noqa: F401
    import concourse.mybir as mybir
    import concourse.tile as tile
    from concourse.bacc import Bacc

    f32 = mybir.dt.float32
    w_dt = getattr(mybir.dt, w_dt_name)
    qk_dt = getattr(mybir.dt, qk_dt_name)
    mask_dt = getattr(mybir.dt, mask_dt_name)

    kt_n = lk // 128  # k-tiles
    ns_n = lq // 512  # q slices of 512
    assert lk % 128 == 0 and lq % 512 == 0

    nc = Bacc(name="dotatt")

    # Stacked hi/lo bf16 decomposition of q and k. Contract dim is 2*d=128
    # (full PE array). Two bf16 matmuls per output tile compute all four
    # hi/lo cross terms, giving near-fp32 scores at bf16 matmul rate:
    #   kq = [K^ | Q^A | Q^B],  K^ = [hi_k; lo_k],
    #   Q^A = [hi_q; lo_q]  (terms hh + ll),  Q^B = [lo_q; hi_q]  (hl + lh)
    bf16 = mybir.dt.bfloat16
    kqh = nc.dram_tensor("kqh", [bpc, 2 * d, lk + 2 * lq], bf16,
                         kind="ExternalInput")
    vp = nc.dram_tensor("vp", [bpc, 128, kt_n, d], w_dt, kind="ExternalInput")
    mT = nc.dram_tensor("mT", [bpc, kt_n, 128, lq], mask_dt, kind="ExternalInput")
    outT = nc.dram_tensor("outT", [bpc, d, lq], f32, kind="ExternalOutput")

    with tile.TileContext(nc) as tc:
        with (
            tc.tile_pool(name="io", bufs=1) as io,
            tc.tile_pool(name="wp", bufs=kt_n + 6) as wp,
            tc.tile_pool(name="mp", bufs=6) as mp,
            tc.tile_pool(name="st", bufs=4) as st,
            tc.tile_pool(name="ps", bufs=2, space="PSUM") as ps,
        ):
            # Per-batch emission helpers. Emission order controls the PE
            # queue order; the first PRE tiles of batch b+1 are emitted
            # before batch b's AV phase so DVE has max/exp work during the
            # window where it previously idled behind the in-order AV MMs.
            state = {}

            def emit_dma(b):
                kq_sb = io.tile([2 * d, lk + 2 * lq], bf16, tag="kq",
                                name=f"kq{b}", bufs=2)
                v_sb = io.tile([128, kt_n, d], w_dt, tag="v", name=f"v{b}",
                               bufs=2)
                nc.sync.dma_start(out=kq_sb[:, lk:lk + lq],
                                  in_=kqh[b, :, lk:lk + lq])
                nc.scalar.dma_start(out=kq_sb[:, lk + lq:],
                                    in_=kqh[b, :, lk + lq:])
                for kc in range(4):
                    c0, c1 = kc * (lk // 4), (kc + 1) * (lk // 4)
                    nc.sync.dma_start(out=kq_sb[:, c0:c1], in_=kqh[b, :, c0:c1])
                nc.scalar.dma_start(out=v_sb, in_=vp[b])
                half = kt_n // 2
                state[b] = dict(
                    kq=kq_sb, v=v_sb, wts=[], vps=[],
                    se=[st.tile([128, half], f32, tag="sea", name=f"sea{b}{h}",
                                bufs=4) for h in range(2)],
                    rs=[st.tile([128, half], f32, tag="rsa", name=f"rsa{b}{h}",
                                bufs=4) for h in range(2)],
                )

            def emit_tile(b, kt):
                st_b = state[b]
                kq_sb = st_b["kq"]
                half = kt_n // 2
                s_ps = ps.tile([128, lq], f32, tag="big", name=f"s{b}_{kt}")
                lhs = kq_sb[:, kt * 128:(kt + 1) * 128]
                for n in range(ns_n):
                    nc.tensor.matmul(
                        s_ps[:, n * 512:(n + 1) * 512], lhs,
                        kq_sb[:, lk + n * 512:lk + (n + 1) * 512],
                        start=True, stop=False)
                    nc.tensor.matmul(
                        s_ps[:, n * 512:(n + 1) * 512], lhs,
                        kq_sb[:, lk + lq + n * 512:lk + lq + (n + 1) * 512],
                        start=False, stop=True)
                negmax = st.tile([128, 1], f32, tag="nm", name=f"nm{b}_{kt}")
                nc.vector.reduce_max(
                    negmax, s_ps, axis=mybir.AxisListType.X, negate=True)
                w_sb = wp.tile([128, lq], w_dt, tag="w", name=f"w{b}_{kt}")
                h, j = divmod(kt, half)
                nc.scalar.activation(
                    w_sb, s_ps, mybir.ActivationFunctionType.Exp,
                    bias=negmax, scale=1.0,
                    accum_out=st_b["se"][h][:, j:j + 1])
                st_b["wts"].append(w_sb)
                if j == half - 1:
                    nc.vector.reciprocal(st_b["rs"][h], st_b["se"][h])
                # mask multiply (in-place, bf16 2x); DMA rings alternate
                m_sb = mp.tile([128, lq], mask_dt, tag="m", name=f"m{b}_{kt}")
                deng = nc.sync if kt % 2 == 0 else nc.scalar
                deng.dma_start(out=m_sb, in_=mT[b, kt])
                nc.vector.tensor_tensor(
                    out=w_sb, in0=w_sb, in1=m_sb, op=mybir.AluOpType.mult)

            def emit_av(b):
                st_b = state[b]
                half = kt_n // 2
                for kt in range(kt_n):
                    h, j = divmod(kt, half)
                    vk = st.tile([128, d], w_dt, tag="vk", name=f"vk{b}_{kt}",
                                 bufs=kt_n + 2)
                    nc.scalar.activation(
                        vk, st_b["v"][:, kt],
                        mybir.ActivationFunctionType.Copy,
                        bias=0.0, scale=st_b["rs"][h][:, j:j + 1])
                    st_b["vps"].append(vk)
                o_ps = ps.tile([d, lq], f32, tag="big", name=f"o{b}")
                for kt in range(kt_n):
                    for n in range(ns_n):
                        nc.tensor.matmul(
                            o_ps[:, n * 512:(n + 1) * 512],
                            st_b["vps"][kt],
                            st_b["wts"][kt][:, n * 512:(n + 1) * 512],
                            start=(kt == 0), stop=(kt == kt_n - 1))
                o_sb = io.tile([d, lq], f32, tag="o", name=f"o{b}", bufs=2)
                for n in range(ns_n):
                    nc.scalar.copy(
                        o_sb[:, n * 512:(n + 1) * 512],
                        o_ps[:, n * 512:(n + 1) * 512])
                nc.scalar.dma_start(out=outT[b], in_=o_sb)

            PRE = 6
            for b in range(bpc):
                emit_dma(b)
                for kt in range(kt_n):
                    emit_tile(b, kt)
                    # after PRE tiles of batch b, flush the previous batch's AV
                    if kt == PRE - 1 and b > 0:
                        emit_av(b - 1)
            emit_av(bpc - 1)
    nc.compile()
    return nc


def make_mask_full():
    """Bit-exact reproduction of the reference dropout keep-mask on CPU."""
    import jax

    with jax.default_device(jax.devices("cpu")[0]):
        keep = jax.random.bernoulli(jax.random.key(42), 1.0 - DROP_P, (B, LQ, LK))
        return np.asarray(keep)


def prep_inputs(q_mat, k_mat, v_mat, keep):
    """Host-side shard + layout prep. Returns list of per-core input dicts."""
    import ml_dtypes

    q = np.asarray(q_mat, dtype=np.float32) * np.float32(SCALE)
    k = np.asarray(k_mat, dtype=np.float32)
    v = np.asarray(v_mat, dtype=np.float32) / np.float32(1.0 - DROP_P)

    kt_n = LK // 128
    in_maps = []
    for c in range(N_CORES):
        sl = slice(c * BPC, (c + 1) * BPC)
        qT = q[sl].transpose(0, 2, 1)  # [bpc, D, Lq], already *64
        kT = k[sl].transpose(0, 2, 1)  # [bpc, D, Lk]

        def hilo(x):
            hi = x.astype(ml_dtypes.bfloat16)
            lo = (x - hi.astype(np.float32)).astype(ml_dtypes.bfloat16)
            return hi, lo

        hq, lq_ = hilo(qT)
        hk, lk_ = hilo(kT)
        khat = np.concatenate([hk, lk_], axis=1)   # [bpc, 2D, Lk]
        qhatA = np.concatenate([hq, lq_], axis=1)  # [bpc, 2D, Lq]
        qhatB = np.concatenate([lq_, hq], axis=1)
        kqh = np.ascontiguousarray(
            np.concatenate([khat, qhatA, qhatB], axis=2)
        )  # [bpc, 2D, Lk+2Lq]
        np_w_dt = ml_dtypes.bfloat16 if W_DT == "bfloat16" else np.float32
        vp = np.ascontiguousarray(
            v[sl].reshape(BPC, kt_n, 128, D).transpose(0, 2, 1, 3)
        ).astype(np_w_dt)  # [bpc, 128, kt, D]
        # mask transposed to [k, q], packed per k-tile, values {0, 1}
        mt = keep[sl].transpose(0, 2, 1).reshape(BPC, kt_n, 128, LQ)
        np_mask_dt = {
            "bfloat16": ml_dtypes.bfloat16,
            "float8e4": ml_dtypes.float8_e4m3,
            "float32": np.float32,
        }[MASK_DT]
        m = mt.astype(np_mask_dt)
        in_maps.append({"kqh": kqh, "vp": vp, "mT": np.ascontiguousarray(m)})
    return in_maps


_CACHE = {}


def kernel(q_mat, k_mat, v_mat, _trace=False):
    from concourse import bass_utils

    if "nc" not in _CACHE:
        _CACHE["nc"] = build_nc()
        _CACHE["keep"] = make_mask_full()
    nc = _CACHE["nc"]
    in_maps = prep_inputs(q_mat, k_mat, v_mat, _CACHE["keep"])

    res = bass_utils.run_bass_kernel_spmd(
        nc, in_maps, core_ids=list(range(N_CORES)), trace=_trace
    )
    _CACHE["last_results"] = res

    out = np.empty((B, LQ, D), dtype=np.float32)
    for c in range(N_CORES):
        oT = res.results[c]["outT"]  # [bpc, D, Lq]
        for i in range(BPC):
            out[c * BPC + i] = oT[i].T
    return out


# revision 37
# speedup vs baseline: 1.0308x; 1.0308x over previous
"""Trainium2 Bass kernel for nn_DotAtt_10179072492025.

reference:
    scores  = einsum('bqd,bkd->bqk', q, k) * 64.0        # [B, Lq, Lk]
    weights = softmax(scores, axis=-2)                   # over the QUERY axis
    keep    = bernoulli(key(42), 0.9, weights.shape)
    weights = where(keep, weights / 0.9, 0)
    out     = einsum('bqk,bkd->bqd', weights, v)         # [B, Lq, D]

Strategy (B=16 sharded across 8 cores, 2 batches/core, no cross-core comm):
  Work in the TRANSPOSED score layout S_T[k, q] so the softmax reduction axis
  (q) is the free axis.  Scores need ~fp32 precision (SCALE=64 makes exp
  amplify any score error), but fp32 matmul is 4 cyc/row and float32r is only
  ~12-bit on HW; instead q and k are split hi/lo in bf16 and stacked along
  the contract dim (2*64=128, full PE array).  Two bf16 matmuls per output
  slice yield all four cross terms:
    MM1: [hi_k;lo_k]^T @ [hi_q;lo_q]  = hh + ll
    MM2: [hi_k;lo_k]^T @ [lo_q;hi_q]  = hl + lh
  Per k-tile of 128 rows:
    PE  : S_T[128,2048] psum = the two matmuls per 512-slice (accumulated)
    DVE : negmax[k] = -max_q S_T               (reduce_max, negate)
    ACT : W[k,q] = exp(S_T + negmax) -> bf16, sum[k] = accum_out (one op)
    DVE : W *= mask (in-place bf16 tensor_tensor, 2x perf mode)
    ACT : v'[k,:] = v[k,:] * (1/sum[k])        (Copy activation, scale=AP)
    PE  : outT[d,q] += v'.T @ W                (phase 2, accumulate over k)
  Host side: q pre-scaled by 64 (exact, pow2), v pre-scaled by 1/0.9 and cast
  bf16, dropout mask from jax threefry on CPU (bit-exact with the reference),
  transposed to [k,q] bf16 {0,1}; output returned transposed and fixed on host.
  Built with bacc.Bacc (its compile() legalizes to <=1 sync wait per
  instruction, a TRN2 hardware constraint this walrus build enforces).
"""

import os
import sys

if "/opt/trn_rl_repo" not in sys.path:
    sys.path.insert(0, "/opt/trn_rl_repo")

import numpy as np

B, LQ, LK, D = 16, 2048, 2048, 64
SCALE = 64.0
DROP_P = 0.1
N_CORES = 8
BPC = B // N_CORES  # batches per core

# tunables (env-overridable for A/B testing)
W_DT = os.environ.get("DOTATT_W_DT", "bfloat16")  # weight tile dtype
MASK_DT = os.environ.get("DOTATT_MASK_DT", "bfloat16")  # mask dram dtype
MASK_ENGINE = os.environ.get("DOTATT_MASK_ENGINE", "gpsimd")  # gpsimd|vector
QK_DT = os.environ.get("DOTATT_QK_DT", "float32r")  # float32r | float32


def build_nc(bpc=BPC, lq=LQ, lk=LK, d=D, w_dt_name=W_DT, mask_dt_name=MASK_DT,
             qk_dt_name=QK_DT, mask_engine=MASK_ENGINE):
    """Build the Bass/Tile program (SPMD: same program on every core)."""
    import concourse.bass as bass  # noqa: F401
    import concourse.mybir as mybir
    import concourse.tile as tile
    from concourse.bacc import Bacc

    f32 = mybir.dt.float32
    w_dt = getattr(mybir.dt, w_dt_name)
    qk_dt = getattr(mybir.dt, qk_dt_name)
    mask_dt = getattr(mybir.dt, mask_dt_name)

    kt_n = lk // 128  # k-tiles
    ns_n = lq // 512  # q slices of 512
    assert lk % 128 == 0 and lq % 512 == 0

    nc = Bacc(name="dotatt")

    # Stacked hi/lo bf16 decomposition of q and k. Contract dim is 2*d=128
    # (full PE array). Two bf16 matmuls per output tile compute all four
    # hi/lo cross terms, giving near-fp32 scores at bf16 matmul rate:
    #   kq = [K^ | Q^A | Q^B],  K^ = [hi_k; lo_k],
    #   Q^A = [hi_q; lo_q]  (terms hh + ll),  Q^B = [lo_q; hi_q]  (hl + lh)
    bf16 = mybir.dt.bfloat16
    kqh = nc.dram_tensor("kqh", [bpc, 2 * d, lk + 2 * lq], bf16,
                         kind="ExternalInput")
    vp = nc.dram_tensor("vp", [bpc, 128, kt_n, d], w_dt, kind="ExternalInput")
    mT = nc.dram_tensor("mT", [bpc, kt_n, 128, lq], mask_dt, kind="ExternalInput")
    outT = nc.dram_tensor("outT", [bpc, d, lq], f32, kind="ExternalOutput")

    with tile.TileContext(nc) as tc:
        with (
            tc.tile_pool(name="io", bufs=1) as io,
            tc.tile_pool(name="wp", bufs=kt_n + 6) as wp,
            tc.tile_pool(name="mp", bufs=6) as mp,
            tc.tile_pool(name="st", bufs=4) as st,
            tc.tile_pool(name="ps", bufs=2, space="PSUM") as ps,
        ):
            # Per-batch emission helpers. Emission order controls the PE
            # queue order; the first PRE tiles of batch b+1 are emitted
            # before batch b's AV phase so DVE has max/exp work during the
            # window where it previously idled behind the in-order AV MMs.
            state = {}

            def emit_dma(b):
                kq_sb = io.tile([2 * d, lk + 2 * lq], bf16, tag="kq",
                                name=f"kq{b}", bufs=2)
                v_sb = io.tile([128, kt_n, d], w_dt, tag="v", name=f"v{b}",
                               bufs=2)
                nc.sync.dma_start(out=kq_sb[:, lk:lk + lq],
                                  in_=kqh[b, :, lk:lk + lq])
                nc.scalar.dma_start(out=kq_sb[:, lk + lq:],
                                    in_=kqh[b, :, lk + lq:])
                for kc in range(4):
                    c0, c1 = kc * (lk // 4), (kc + 1) * (lk // 4)
                    nc.sync.dma_start(out=kq_sb[:, c0:c1], in_=kqh[b, :, c0:c1])
                nc.scalar.dma_start(out=v_sb, in_=vp[b])
                half = kt_n // 2
                state[b] = dict(
                    kq=kq_sb, v=v_sb, wts=[], vps=[],
                    se=[st.tile([128, half], f32, tag="sea", name=f"sea{b}{h}",
                                bufs=4) for h in range(2)],
                    rs=[st.tile([128, half], f32, tag="rsa", name=f"rsa{b}{h}",
                                bufs=4) for h in range(2)],
                )

            def emit_tile(b, kt):
                st_b = state[b]
                kq_sb = st_b["kq"]
                half = kt_n // 2
                s_ps = ps.tile([128, lq], f32, tag="big", name=f"s{b}_{kt}")
                lhs = kq_sb[:, kt * 128:(kt + 1) * 128]
                for n in range(ns_n):
                    nc.tensor.matmul(
                        s_ps[:, n * 512:(n + 1) * 512], lhs,
                        kq_sb[:, lk + n * 512:lk + (n + 1) * 512],
                        start=True, stop=False)
                    nc.tensor.matmul(
                        s_ps[:, n * 512:(n + 1) * 512], lhs,
                        kq_sb[:, lk + lq + n * 512:lk + lq + (n + 1) * 512],
                        start=False, stop=True)
                negmax = st.tile([128, 1], f32, tag="nm", name=f"nm{b}_{kt}")
                nc.vector.reduce_max(
                    negmax, s_ps, axis=mybir.AxisListType.X, negate=True)
                w_sb = wp.tile([128, lq], w_dt, tag="w", name=f"w{b}_{kt}")
                h, j = divmod(kt, half)
                nc.scalar.activation(
                    w_sb, s_ps, mybir.ActivationFunctionType.Exp,
                    bias=negmax, scale=1.0,
                    accum_out=st_b["se"][h][:, j:j + 1])
                st_b["wts"].append(w_sb)
                if j == half - 1:
                    nc.vector.reciprocal(st_b["rs"][h], st_b["se"][h])
                # mask multiply (in-place, bf16 2x); DMA rings alternate
                m_sb = mp.tile([128, lq], mask_dt, tag="m", name=f"m{b}_{kt}")
                deng = nc.sync if kt % 2 == 0 else nc.scalar
                deng.dma_start(out=m_sb, in_=mT[b, kt])
                nc.vector.tensor_tensor(
                    out=w_sb, in0=w_sb, in1=m_sb, op=mybir.AluOpType.mult)

            def emit_av(b):
                st_b = state[b]
                half = kt_n // 2
                for kt in range(kt_n):
                    h, j = divmod(kt, half)
                    vk = st.tile([128, d], w_dt, tag="vk", name=f"vk{b}_{kt}",
                                 bufs=kt_n + 2)
                    nc.scalar.activation(
                        vk, st_b["v"][:, kt],
                        mybir.ActivationFunctionType.Copy,
                        bias=0.0, scale=st_b["rs"][h][:, j:j + 1])
                    st_b["vps"].append(vk)
                o_ps = ps.tile([d, lq], f32, tag="big", name=f"o{b}")
                for kt in range(kt_n):
                    for n in range(ns_n):
                        nc.tensor.matmul(
                            o_ps[:, n * 512:(n + 1) * 512],
                            st_b["vps"][kt],
                            st_b["wts"][kt][:, n * 512:(n + 1) * 512],
                            start=(kt == 0), stop=(kt == kt_n - 1))
                o_sb = io.tile([d, lq], f32, tag="o", name=f"o{b}", bufs=2)
                for n in range(ns_n):
                    nc.scalar.copy(
                        o_sb[:, n * 512:(n + 1) * 512],
                        o_ps[:, n * 512:(n + 1) * 512])
                nc.sync.dma_start(out=outT[b], in_=o_sb)

            PRE = 6
            for b in range(bpc):
                emit_dma(b)
                for kt in range(kt_n):
                    emit_tile(b, kt)
                    # after PRE tiles of batch b, flush the previous batch's AV
                    if kt == PRE - 1 and b > 0:
                        emit_av(b - 1)
            emit_av(bpc - 1)
    nc.compile()
    return nc


def make_mask_full():
    """Bit-exact reproduction of the reference dropout keep-mask on CPU."""
    import jax

    with jax.default_device(jax.devices("cpu")[0]):
        keep = jax.random.bernoulli(jax.random.key(42), 1.0 - DROP_P, (B, LQ, LK))
        return np.asarray(keep)


def prep_inputs(q_mat, k_mat, v_mat, keep):
    """Host-side shard + layout prep. Returns list of per-core input dicts."""
    import ml_dtypes

    q = np.asarray(q_mat, dtype=np.float32) * np.float32(SCALE)
    k = np.asarray(k_mat, dtype=np.float32)
    v = np.asarray(v_mat, dtype=np.float32) / np.float32(1.0 - DROP_P)

    kt_n = LK // 128
    in_maps = []
    for c in range(N_CORES):
        sl = slice(c * BPC, (c + 1) * BPC)
        qT = q[sl].transpose(0, 2, 1)  # [bpc, D, Lq], already *64
        kT = k[sl].transpose(0, 2, 1)  # [bpc, D, Lk]

        def hilo(x):
            hi = x.astype(ml_dtypes.bfloat16)
            lo = (x - hi.astype(np.float32)).astype(ml_dtypes.bfloat16)
            return hi, lo

        hq, lq_ = hilo(qT)
        hk, lk_ = hilo(kT)
        khat = np.concatenate([hk, lk_], axis=1)   # [bpc, 2D, Lk]
        qhatA = np.concatenate([hq, lq_], axis=1)  # [bpc, 2D, Lq]
        qhatB = np.concatenate([lq_, hq], axis=1)
        kqh = np.ascontiguousarray(
            np.concatenate([khat, qhatA, qhatB], axis=2)
        )  # [bpc, 2D, Lk+2Lq]
        np_w_dt = ml_dtypes.bfloat16 if W_DT == "bfloat16" else np.float32
        vp = np.ascontiguousarray(
            v[sl].reshape(BPC, kt_n, 128, D).transpose(0, 2, 1, 3)
        ).astype(np_w_dt)  # [bpc, 128, kt, D]
        # mask transposed to [k, q], packed per k-tile, values {0, 1}
        mt = keep[sl].transpose(0, 2, 1).reshape(BPC, kt_n, 128, LQ)
        np_mask_dt = {
            "bfloat16": ml_dtypes.bfloat16,
            "float8e4": ml_dtypes.float8_e4m3,
            "float32": np.float32,
        }[MASK_DT]
        m = mt.astype(np_mask_dt)
        in_maps.append({"kqh": kqh, "vp": vp, "mT": np.ascontiguousarray(m)})
    return in_maps


_CACHE = {}


def kernel(q_mat, k_mat, v_mat, _trace=False):
    from concourse import bass_utils

    if "nc" not in _CACHE:
        _CACHE["nc"] = build_nc()
        _CACHE["keep"] = make_mask_full()
    nc = _CACHE["nc"]
    in_maps = prep_inputs(q_mat, k_mat, v_mat, _CACHE["keep"])

    res = bass_utils.run_bass_kernel_spmd(
        nc, in_maps, core_ids=list(range(N_CORES)), trace=_trace
    )
    _CACHE["last_results"] = res

    out = np.empty((B, LQ, D), dtype=np.float32)
    for c in range(N_CORES):
        oT = res.results[c]["outT"]  # [bpc, D, Lq]
        for i in range(BPC):
            out[c * BPC + i] = oT[i].T
    return out
